# revision 8
# baseline (speedup 1.0000x reference)
"""Trainium2 Bass kernel for the soft-decision-tree ensemble classifier.

Math (per batch row b, tree t):
  zb[t,n]      = x[b] . W[t,n] + bias[t,n]
  log s        = zb - softplus(zb);  log(1-s) = -softplus(zb)
  log_leaf[l]  = sum_{k in path(l)} dir_k * zb_k  -  sum_{k in path(l)} softplus(zb_k)
  leaf_prob    = exp(log_leaf)
  out[b,c]     = sum_t 2*softmax(tw)_t * sum_l leaf_prob[t,l] * softmax(leaf_logits[t,l])_c

Mapping: data-parallel over the batch (B=4096 -> 512 rows per NeuronCore).
Per core, logits live in [tree-node (padded 64/tree), batch] layout so the
per-tree path sums become 128-wide matmuls with +/-1 constant matrices
(block-diagonal over a pair of trees per 128-partition tile). All matmuls
run as float32r (full PE rate at free-dim 512). softplus = Ln(Exp(x)+1) so
the whole kernel uses a single ACT function table (exp/ln).
"""

import numpy as np

TREE_DEPTH = 6
T, N, D, C = 64, 63, 512, 100
L = 2**TREE_DEPTH          # 64
NPAD = 64                  # nodes padded per tree
TNP = T * NPAD             # 4096
NTILES = TNP // 128        # 32 (two trees per 128-partition tile)
B = 4096
NCORES = 8
BS = B // NCORES           # 512


def _leaf_paths(depth):
    Ll = 2**depth
    idx = np.zeros((Ll, depth), np.int32)
    dr = np.zeros((Ll, depth), np.int32)
    for l in range(Ll):
        node = 0
        for k in range(depth):
            bit = (l >> (depth - 1 - k)) & 1
            idx[l, k] = node
            dr[l, k] = bit
            node = 2 * node + 1 + bit
    return idx, dr


def _host_consts():
    idx, dr = _leaf_paths(TREE_DEPTH)
    mdir = np.zeros((NPAD, L), np.float32)   # [node, leaf] +1 where dir=1
    mpath = np.zeros((NPAD, L), np.float32)  # [node, leaf] -1 on path
    for l in range(L):
        for k in range(TREE_DEPTH):
            n = idx[l, k]
            mpath[n, l] -= 1.0
            if dr[l, k]:
                mdir[n, l] += 1.0
    adir = np.zeros((128, 128), np.float32)
    apath = np.zeros((128, 128), np.float32)
    adir[:NPAD, :L] = mdir
    adir[NPAD:, L:] = mdir
    apath[:NPAD, :L] = mpath
    apath[NPAD:, L:] = mpath
    # a2[t, p] = 1 if (t % 2) == p // 64 ; e2[t, i] = 1 if t // 2 == i
    a2 = np.zeros((T, 128), np.float32)
    a2[0::2, :64] = 0.0
    for t in range(T):
        a2[t, (t % 2) * 64:(t % 2) * 64 + 64] = 1.0
    e2 = np.zeros((T, NTILES), np.float32)
    for t in range(T):
        e2[t, t // 2] = 1.0
    return adir, apath, a2, e2


_NC_CACHE = {}


def _build_bass():
    import concourse.bacc as bacc
    import concourse.mybir as mybir
    import concourse.tile as tile
    from concourse.masks import make_identity

    dt = mybir.dt
    f32 = dt.float32
    f32r = dt.float32r
    bf16 = dt.bfloat16
    AF = mybir.ActivationFunctionType
    ALU = mybir.AluOpType
    AX = mybir.AxisListType

    nc = bacc.Bacc("TRN2", target_bir_lowering=False, debug=False,
                   num_devices=NCORES)

    # Pin the ACT function table to one containing BOTH Exp and Ln, else the
    # table-load pass ping-pongs between single-function tables (~1.3us per
    # reload, one per activation).
    from concourse.hw_specs import get_activation_tables
    AFT = mybir.ActivationFunctionType
    table_id = next(i for i, (_, funcs) in
                    enumerate(get_activation_tables("gen3").items())
                    if AFT.Exp in funcs and AFT.Ln in funcs)
    nc.scalar.add_instruction(mybir.InstLoadActFuncSet(
        name=f"I-{nc.next_id()}", ins=[], outs=[], act_func_set_id=table_id))

    xt = nc.dram_tensor("xt", [D, BS], bf16, kind="ExternalInput").ap()
    wt = nc.dram_tensor("wt", [D, TNP], bf16, kind="ExternalInput").ap()
    biasc = nc.dram_tensor("biasc", [128, NTILES], f32, kind="ExternalInput").ap()
    llf = nc.dram_tensor("llf", [TNP, C], f32, kind="ExternalInput").ap()
    tw = nc.dram_tensor("tw", [1, T], f32, kind="ExternalInput").ap()
    adir = nc.dram_tensor("adir", [128, 128], f32r, kind="ExternalInput").ap()
    apath = nc.dram_tensor("apath", [128, 128], f32r, kind="ExternalInput").ap()
    a2 = nc.dram_tensor("a2", [T, 128], f32, kind="ExternalInput").ap()
    e2 = nc.dram_tensor("e2", [T, NTILES], f32, kind="ExternalInput").ap()
    out = nc.dram_tensor("out", [C, BS], f32, kind="ExternalOutput").ap()

    with tile.TileContext(nc) as tc:
        with (
            tc.tile_pool(name="big", bufs=1) as bigp,
            tc.tile_pool(name="const", bufs=1) as constp,
            tc.tile_pool(name="work", bufs=3) as work,
            tc.tile_pool(name="tmp", bufs=2) as tmpp,
            tc.tile_pool(name="ps", bufs=2, space="PSUM") as psp,
            tc.tile_pool(name="ps1", bufs=1, space="PSUM") as ps1,
        ):
            # ---- persistent loads -------------------------------------
            wt_t = []
            for j in range(4):
                wtile = bigp.tile([128, TNP], bf16, tag=f"wt{j}")
                wt_t.append(wtile)
            for q in range(4):
                cs = q * (TNP // 4)
                ce = cs + TNP // 4
                for j in range(4):
                    nc.sync.dma_start(out=wt_t[j][:, cs:ce],
                                      in_=wt[j * 128:(j + 1) * 128, cs:ce])
            xt_t = []
            for j in range(4):
                xtile = bigp.tile([128, BS], bf16, tag=f"xt{j}")
                nc.sync.dma_start(out=xtile[:], in_=xt[j * 128:(j + 1) * 128, :])
                xt_t.append(xtile)
            ll_t = bigp.tile([128, NTILES * C], f32, tag="ll")
            nc.sync.dma_start(
                out=ll_t[:].rearrange("p (i c) -> p i c", c=C),
                in_=llf.rearrange("(i p) c -> p i c", p=128),
            )
            biasc_t = constp.tile([128, NTILES], f32, tag="biasc")
            nc.sync.dma_start(out=biasc_t[:], in_=biasc[:])
            adir_t = constp.tile([128, 128], f32r, tag="adir")
            nc.sync.dma_start(out=adir_t[:], in_=adir[:])
            apath_t = constp.tile([128, 128], f32r, tag="apath")
            nc.sync.dma_start(out=apath_t[:], in_=apath[:])
            a2_t = constp.tile([T, 128], f32, tag="a2")
            nc.sync.dma_start(out=a2_t[:], in_=a2[:])
            e2_t = constp.tile([T, NTILES], f32, tag="e2")
            nc.sync.dma_start(out=e2_t[:], in_=e2[:])
            tw_t = constp.tile([1, T], f32, tag="tw")
            nc.sync.dma_start(out=tw_t[:], in_=tw[:])

            ident = constp.tile([64, 64], f32, tag="ident")
            make_identity(nc, ident[:])

            # ---- tree-weight softmax -> per-partition scale columns ----
            mneg = constp.tile([1, 1], f32, tag="mneg")
            nc.vector.tensor_reduce(out=mneg[:], in_=tw_t[:], op=ALU.max,
                                    axis=AX.X, negate=True)
            ew = constp.tile([1, T], f32, tag="ew")
            nc.scalar.activation(ew[:], tw_t[:], AF.Exp, bias=mneg[:, 0:1],
                                 scale=1.0)
            sw = constp.tile([1, 1], f32, tag="sw")
            nc.vector.tensor_reduce(out=sw[:], in_=ew[:], op=ALU.add, axis=AX.X)
            rw = constp.tile([1, 1], f32, tag="rw")
            nc.vector.reciprocal(rw[:], sw[:])
            wrow = constp.tile([1, T], f32, tag="wrow")
            nc.vector.tensor_scalar(out=wrow[:], in0=ew[:], scalar1=rw[:, 0:1],
                                    scalar2=2.0, op0=ALU.mult, op1=ALU.mult)
            wcol_ps = ps1.tile([T, 1], f32, tag="wcolps")
            nc.tensor.transpose(wcol_ps[:], wrow[:], ident[0:1, 0:1])
            wcol = constp.tile([T, 1], f32, tag="wcol")
            nc.vector.tensor_copy(out=wcol[:], in_=wcol_ps[:])
            bmat = constp.tile([T, NTILES], f32, tag="bmat")
            nc.vector.tensor_scalar_mul(bmat[:], e2_t[:], wcol[:, 0:1])
            w2_ps = ps1.tile([128, NTILES], f32, tag="w2ps")
            nc.tensor.matmul(w2_ps[:], lhsT=a2_t[:], rhs=bmat[:],
                             start=True, stop=True)
            w2c = constp.tile([128, NTILES], f32, tag="w2c")
            nc.vector.tensor_copy(out=w2c[:], in_=w2_ps[:])

            # ---- leaf distributions: one big exp + rowsum ------------
            ev_all = bigp.tile([128, NTILES * C], f32, tag="evall")
            nc.scalar.activation(ev_all[:], ll_t[:], AF.Exp)
            sv_all = constp.tile([128, NTILES], f32, tag="svall")
            nc.vector.tensor_reduce(
                out=sv_all[:],
                in_=ev_all[:].rearrange("p (i c) -> p i c", c=C),
                op=ALU.add, axis=AX.X)
            rv_all = constp.tile([128, NTILES], f32, tag="rvall")
            nc.vector.reciprocal(rv_all[:], sv_all[:])

            # ---- main pipeline ---------------------------------------
            out_ps = ps1.tile([C, BS], f32, tag="outps")
            for i in range(NTILES):
                pz = psp.tile([128, BS], f32, tag="pz")
                for j in range(4):
                    nc.tensor.matmul(
                        pz[:],
                        lhsT=wt_t[j][:, i * 128:(i + 1) * 128],
                        rhs=xt_t[j][:],
                        start=(j == 0), stop=(j == 3),
                    )
                bsl = biasc_t[:, i:i + 1]
                ta = work.tile([128, BS], f32r, tag="ta")
                nc.vector.tensor_scalar_add(out=ta[:], in0=pz[:], scalar1=bsl)
                te = tmpp.tile([128, BS], f32, tag="te")
                nc.scalar.activation(te[:], pz[:], AF.Exp, bias=bsl, scale=1.0)
                tb = work.tile([128, BS], f32r, tag="tb")
                nc.scalar.activation(tb[:], te[:], AF.Ln, bias=1.0, scale=1.0)
                pp = psp.tile([128, BS], f32, tag="pp")
                nc.tensor.matmul(pp[:], lhsT=adir_t[:],
                                 rhs=ta[:], start=True, stop=False)
                nc.tensor.matmul(pp[:], lhsT=apath_t[:],
                                 rhs=tb[:], start=False, stop=True)
                lp = work.tile([128, BS], bf16, tag="lp")
                nc.scalar.activation(lp[:], pp[:], AF.Exp)

                vt = work.tile([128, C], bf16, tag="vt")
                nc.vector.tensor_scalar(out=vt[:],
                                        in0=ev_all[:, i * C:(i + 1) * C],
                                        scalar1=rv_all[:, i:i + 1],
                                        scalar2=w2c[:, i:i + 1],
                                        op0=ALU.mult, op1=ALU.mult)
                nc.tensor.matmul(out_ps[:], lhsT=vt[:],
                                 rhs=lp[:],
                                 start=(i == 0), stop=(i == NTILES - 1))

            out_sb = work.tile([C, BS], f32, tag="osb")
            nc.vector.tensor_copy(out=out_sb[:], in_=out_ps[:])
            nc.sync.dma_start(out=out[:], in_=out_sb[:])

    nc.finalize()
    return nc


def _get_nc():
    if "nc" not in _NC_CACHE:
        _NC_CACHE["nc"] = _build_bass()
    return _NC_CACHE["nc"]


def kernel(x, split_weights, split_bias, leaf_logits, tree_weights):
    from concourse.bass_utils import run_bass_kernel_spmd

    x = np.ascontiguousarray(np.asarray(x, np.float32))
    split_weights = np.asarray(split_weights, np.float32)
    split_bias = np.asarray(split_bias, np.float32)
    leaf_logits = np.asarray(leaf_logits, np.float32)
    tree_weights = np.asarray(tree_weights, np.float32)

    import ml_dtypes
    adir, apath, a2, e2 = _host_consts()  # noqa: ml_dtypes used below

    wpad = np.zeros((T, NPAD, D), np.float32)
    wpad[:, :N, :] = split_weights
    wtT = np.ascontiguousarray(
        wpad.reshape(TNP, D).T.astype(ml_dtypes.bfloat16))      # [D, TNP]
    bpad = np.zeros((T, NPAD), np.float32)
    bpad[:, :N] = split_bias
    biasc = np.ascontiguousarray(bpad.reshape(NTILES, 128).T)   # [128, NTILES]
    llf = np.ascontiguousarray(leaf_logits.reshape(TNP, C))
    tw = np.ascontiguousarray(tree_weights.reshape(1, T))

    shared = dict(wt=wtT, biasc=biasc, llf=llf, tw=tw,
                  adir=adir, apath=apath, a2=a2, e2=e2)
    in_maps = []
    for i in range(NCORES):
        xt = np.ascontiguousarray(
            x[i * BS:(i + 1) * BS, :].T.astype(ml_dtypes.bfloat16))  # [D, BS]
        in_maps.append(dict(xt=xt, **shared))

    nc = _get_nc()
    res = run_bass_kernel_spmd(nc, in_maps, core_ids=list(range(NCORES)))
    out = np.concatenate([res.results[i]["out"] for i in range(NCORES)],
                         axis=1).T                              # [B, C]
    return np.ascontiguousarray(out.astype(np.float32))


# revision 10
# speedup vs baseline: 1.0375x; 1.0375x over previous
"""Trainium2 Bass kernel for the soft-decision-tree ensemble classifier.

Math (per batch row b, tree t):
  zb[t,n]      = x[b] . W[t,n] + bias[t,n]
  log s        = zb - softplus(zb);  log(1-s) = -softplus(zb)
  log_leaf[l]  = sum_{k in path(l)} dir_k * zb_k  -  sum_{k in path(l)} softplus(zb_k)
  leaf_prob    = exp(log_leaf)
  out[b,c]     = sum_t 2*softmax(tw)_t * sum_l leaf_prob[t,l] * softmax(leaf_logits[t,l])_c

Mapping: data-parallel over the batch (B=4096 -> 512 rows per NeuronCore).
Per core, logits live in [tree-node (padded 64/tree), batch] layout so the
per-tree path sums become 128-wide matmuls with +/-1 constant matrices
(block-diagonal over a pair of trees per 128-partition tile). Stage-1/4
matmuls run in bf16, the log-domain path-sum matmul in float32r. softplus
is computed as Ln(Exp(x)+1) so the whole kernel needs a single ACT function
table (pinned up front - the automatic table chooser would otherwise reload
tables between Exp and Ln constantly).
"""

import numpy as np

TREE_DEPTH = 6
T, N, D, C = 64, 63, 512, 100
L = 2**TREE_DEPTH          # 64
NPAD = 64                  # nodes padded per tree
TNP = T * NPAD             # 4096
NTILES = TNP // 128        # 32 (two trees per 128-partition tile)
B = 4096
NCORES = 8
BS = B // NCORES           # 512

# column layout of the packed constants tensor [128, 512]
_COL_BIAS = 0      # [128, 32]
_COL_ADIR = 32     # [128, 128]
_COL_APATH = 160   # [128, 128]
_COL_A2 = 288      # [64, 128]
_COL_E2 = 416      # [64, 32]
_COL_TW = 448      # [1, 64]
_CONST_COLS = 512


def _leaf_paths(depth):
    Ll = 2**depth
    idx = np.zeros((Ll, depth), np.int32)
    dr = np.zeros((Ll, depth), np.int32)
    for l in range(Ll):
        node = 0
        for k in range(depth):
            bit = (l >> (depth - 1 - k)) & 1
            idx[l, k] = node
            dr[l, k] = bit
            node = 2 * node + 1 + bit
    return idx, dr


def _pack_consts(split_bias, tree_weights):
    """Build the [128, 512] packed constants array (f32 bits)."""
    idx, dr = _leaf_paths(TREE_DEPTH)
    mdir = np.zeros((NPAD, L), np.float32)   # [node, leaf] +1 where dir=1
    mpath = np.zeros((NPAD, L), np.float32)  # [node, leaf] -1 on path
    for l in range(L):
        for k in range(TREE_DEPTH):
            n = idx[l, k]
            mpath[n, l] -= 1.0
            if dr[l, k]:
                mdir[n, l] += 1.0
    consts = np.zeros((128, _CONST_COLS), np.float32)
    # bias columns: bias_pad flattened [(tile, partition)] -> [128, 32]
    bpad = np.zeros((T, NPAD), np.float32)
    bpad[:, :N] = split_bias
    consts[:, _COL_BIAS:_COL_BIAS + NTILES] = bpad.reshape(NTILES, 128).T
    # block-diagonal path matrices (two trees per 128-tile)
    consts[:NPAD, _COL_ADIR:_COL_ADIR + L] = mdir
    consts[NPAD:, _COL_ADIR + L:_COL_ADIR + 128] = mdir
    consts[:NPAD, _COL_APATH:_COL_APATH + L] = mpath
    consts[NPAD:, _COL_APATH + L:_COL_APATH + 128] = mpath
    # a2[t, p] = 1 if (t % 2) == p // 64 — broadcast selector for w2 columns
    a2 = np.zeros((T, 128), np.float32)
    for t in range(T):
        a2[t, (t % 2) * 64:(t % 2) * 64 + 64] = 1.0
    consts[:T, _COL_A2:_COL_A2 + 128] = a2
    # e2[t, i] = 1 if t // 2 == i
    e2 = np.zeros((T, NTILES), np.float32)
    for t in range(T):
        e2[t, t // 2] = 1.0
    consts[:T, _COL_E2:_COL_E2 + NTILES] = e2
    consts[0, _COL_TW:_COL_TW + T] = tree_weights
    return consts


_NC_CACHE = {}


def _build_bass():
    import concourse.bacc as bacc
    import concourse.mybir as mybir
    import concourse.tile as tile
    from concourse.hw_specs import get_activation_tables
    from concourse.masks import make_identity

    dt = mybir.dt
    f32 = dt.float32
    f32r = dt.float32r
    bf16 = dt.bfloat16
    AF = mybir.ActivationFunctionType
    ALU = mybir.AluOpType
    AX = mybir.AxisListType

    nc = bacc.Bacc("TRN2", target_bir_lowering=False, debug=False,
                   num_devices=NCORES)

    # Pin the ACT function table to one containing BOTH Exp and Ln, else the
    # table-load pass ping-pongs between single-function tables (~1.3us per
    # reload, one per activation).
    table_id = next(i for i, (_, funcs) in
                    enumerate(get_activation_tables("gen3").items())
                    if AF.Exp in funcs and AF.Ln in funcs)
    nc.scalar.add_instruction(mybir.InstLoadActFuncSet(
        name=f"I-{nc.next_id()}", ins=[], outs=[], act_func_set_id=table_id))

    xt = nc.dram_tensor("xt", [D, BS], bf16, kind="ExternalInput").ap()
    wt = nc.dram_tensor("wt", [D, TNP], bf16, kind="ExternalInput").ap()
    consts = nc.dram_tensor("consts", [128, _CONST_COLS], f32r,
                            kind="ExternalInput").ap()
    llf = nc.dram_tensor("llf", [TNP, C], f32, kind="ExternalInput").ap()
    out = nc.dram_tensor("out", [C, BS], f32, kind="ExternalOutput").ap()

    with tile.TileContext(nc) as tc:
        with (
            tc.tile_pool(name="big", bufs=1) as bigp,
            tc.tile_pool(name="const", bufs=1) as constp,
            tc.tile_pool(name="work", bufs=3) as work,
            tc.tile_pool(name="tmp", bufs=2) as tmpp,
            tc.tile_pool(name="ps", bufs=2, space="PSUM") as psp,
            tc.tile_pool(name="ps1", bufs=1, space="PSUM") as ps1,
        ):
            # ---- input loads, ordered for earliest PE start -----------
            wt_t = [bigp.tile([128, TNP], bf16, tag=f"wt{j}", name=f"wt{j}")
                    for j in range(4)]
            xt_t = bigp.tile([128, 4 * BS], bf16, tag="xt")
            nc.sync.dma_start(out=wt_t[0][:], in_=wt[0:128, :])
            nc.sync.dma_start(
                out=xt_t[:].rearrange("p (j b) -> p j b", b=BS),
                in_=xt.rearrange("(j p) b -> p j b", p=128),
            )
            for j in range(1, 4):
                nc.sync.dma_start(out=wt_t[j][:],
                                  in_=wt[j * 128:(j + 1) * 128, :])
            consts_t = constp.tile([128, _CONST_COLS], f32r, tag="consts")
            nc.sync.dma_start(out=consts_t[:], in_=consts[:])
            ll_t = bigp.tile([128, NTILES * C], f32, tag="ll")
            nc.sync.dma_start(
                out=ll_t[:].rearrange("p (i c) -> p i c", c=C),
                in_=llf.rearrange("(i p) c -> p i c", p=128),
            )

            adir_ap = consts_t[:, _COL_ADIR:_COL_ADIR + 128]
            apath_ap = consts_t[:, _COL_APATH:_COL_APATH + 128]
            a2_ap = consts_t[0:T, _COL_A2:_COL_A2 + 128].bitcast(f32)
            e2_ap = consts_t[0:T, _COL_E2:_COL_E2 + NTILES].bitcast(f32)
            tw_ap = consts_t[0:1, _COL_TW:_COL_TW + T].bitcast(f32)

            def bias_ap(i):
                return consts_t[:, _COL_BIAS + i:_COL_BIAS + i + 1].bitcast(f32)

            ident = constp.tile([64, 64], f32, tag="ident")
            make_identity(nc, ident[:])

            # ---- tree-weight softmax -> per-partition scale columns ----
            mneg = constp.tile([1, 1], f32, tag="mneg")
            nc.vector.tensor_reduce(out=mneg[:], in_=tw_ap, op=ALU.max,
                                    axis=AX.X, negate=True)
            ew = constp.tile([1, T], f32, tag="ew")
            nc.scalar.activation(ew[:], tw_ap, AF.Exp, bias=mneg[:, 0:1],
                                 scale=1.0)
            sw = constp.tile([1, 1], f32, tag="sw")
            nc.vector.tensor_reduce(out=sw[:], in_=ew[:], op=ALU.add, axis=AX.X)
            rw = constp.tile([1, 1], f32, tag="rw")
            nc.vector.reciprocal(rw[:], sw[:])
            wrow = constp.tile([1, T], f32, tag="wrow")
            nc.vector.tensor_scalar(out=wrow[:], in0=ew[:], scalar1=rw[:, 0:1],
                                    scalar2=2.0, op0=ALU.mult, op1=ALU.mult)
            wcol_ps = ps1.tile([T, 1], f32, tag="wcolps")
            nc.tensor.transpose(wcol_ps[:], wrow[:], ident[0:1, 0:1])
            wcol = constp.tile([T, 1], f32, tag="wcol")
            nc.vector.tensor_copy(out=wcol[:], in_=wcol_ps[:])
            bmat = constp.tile([T, NTILES], f32, tag="bmat")
            nc.vector.tensor_scalar_mul(bmat[:], e2_ap, wcol[:, 0:1])
            w2_ps = ps1.tile([128, NTILES], f32, tag="w2ps")
            nc.tensor.matmul(w2_ps[:], lhsT=a2_ap, rhs=bmat[:],
                             start=True, stop=True)
            w2c = constp.tile([128, NTILES], f32, tag="w2c")
            nc.vector.tensor_copy(out=w2c[:], in_=w2_ps[:])

            # ---- leaf distributions: one big exp + rowsum ------------
            ev_all = bigp.tile([128, NTILES * C], f32, tag="evall")
            nc.scalar.activation(ev_all[:], ll_t[:], AF.Exp)
            sv_all = constp.tile([128, NTILES], f32, tag="svall")
            nc.vector.tensor_reduce(
                out=sv_all[:],
                in_=ev_all[:].rearrange("p (i c) -> p i c", c=C),
                op=ALU.add, axis=AX.X)
            rv_all = constp.tile([128, NTILES], f32, tag="rvall")
            nc.vector.reciprocal(rv_all[:], sv_all[:])

            # ---- main pipeline ---------------------------------------
            out_ps = ps1.tile([C, BS], f32, tag="outps")
            for i in range(NTILES):
                pz = psp.tile([128, BS], f32, tag="pz")
                for j in range(4):
                    nc.tensor.matmul(
                        pz[:],
                        lhsT=wt_t[j][:, i * 128:(i + 1) * 128],
                        rhs=xt_t[:, j * BS:(j + 1) * BS],
                        start=(j == 0), stop=(j == 3),
                    )
                bsl = bias_ap(i)
                ta = work.tile([128, BS], f32r, tag="ta")
                nc.vector.tensor_scalar_add(out=ta[:], in0=pz[:], scalar1=bsl)
                te = tmpp.tile([128, BS], f32, tag="te")
                nc.scalar.activation(te[:], pz[:], AF.Exp, bias=bsl, scale=1.0)
                tb = work.tile([128, BS], f32r, tag="tb")
                nc.scalar.activation(tb[:], te[:], AF.Ln, bias=1.0, scale=1.0)
                pp = psp.tile([128, BS], f32, tag="pp")
                nc.tensor.matmul(pp[:], lhsT=adir_ap, rhs=ta[:],
                                 start=True, stop=False)
                nc.tensor.matmul(pp[:], lhsT=apath_ap, rhs=tb[:],
                                 start=False, stop=True)
                lp = work.tile([128, BS], bf16, tag="lp")
                nc.scalar.activation(lp[:], pp[:], AF.Exp)

                vt = work.tile([128, C], bf16, tag="vt")
                nc.vector.tensor_scalar(out=vt[:],
                                        in0=ev_all[:, i * C:(i + 1) * C],
                                        scalar1=rv_all[:, i:i + 1],
                                        scalar2=w2c[:, i:i + 1],
                                        op0=ALU.mult, op1=ALU.mult)
                nc.tensor.matmul(out_ps[:], lhsT=vt[:], rhs=lp[:],
                                 start=(i == 0), stop=(i == NTILES - 1))

            out_sb = work.tile([C, BS], f32, tag="osb")
            nc.vector.tensor_copy(out=out_sb[:], in_=out_ps[:])
            nc.sync.dma_start(out=out[:], in_=out_sb[:])

    nc.finalize()
    return nc


def _get_nc():
    if "nc" not in _NC_CACHE:
        _NC_CACHE["nc"] = _build_bass()
    return _NC_CACHE["nc"]


def _prep_inputs(x, split_weights, split_bias, leaf_logits, tree_weights):
    import ml_dtypes

    x = np.asarray(x, np.float32)
    split_weights = np.asarray(split_weights, np.float32)
    split_bias = np.asarray(split_bias, np.float32)
    leaf_logits = np.asarray(leaf_logits, np.float32)
    tree_weights = np.asarray(tree_weights, np.float32)

    wpad = np.zeros((T, NPAD, D), np.float32)
    wpad[:, :N, :] = split_weights
    wtT = np.ascontiguousarray(
        wpad.reshape(TNP, D).T.astype(ml_dtypes.bfloat16))      # [D, TNP]
    consts = _pack_consts(split_bias, tree_weights)
    llf = np.ascontiguousarray(leaf_logits.reshape(TNP, C))

    shared = dict(wt=wtT, consts=consts, llf=llf)
    in_maps = []
    for i in range(NCORES):
        xti = np.ascontiguousarray(
            x[i * BS:(i + 1) * BS, :].T.astype(ml_dtypes.bfloat16))  # [D, BS]
        in_maps.append(dict(xt=xti, **shared))
    return in_maps


def kernel(x, split_weights, split_bias, leaf_logits, tree_weights):
    from concourse.bass_utils import run_bass_kernel_spmd

    in_maps = _prep_inputs(x, split_weights, split_bias, leaf_logits,
                           tree_weights)
    nc = _get_nc()
    res = run_bass_kernel_spmd(nc, in_maps, core_ids=list(range(NCORES)))
    out = np.concatenate([res.results[i]["out"] for i in range(NCORES)],
                         axis=1).T                              # [B, C]
    return np.ascontiguousarray(out.astype(np.float32))


# revision 11
# speedup vs baseline: 1.0583x; 1.0201x over previous
"""Trainium2 Bass kernel for the soft-decision-tree ensemble classifier.

Math (per batch row b, tree t):
  zb[t,n]      = x[b] . W[t,n] + bias[t,n]
  log s        = zb - softplus(zb);  log(1-s) = -softplus(zb)
  log_leaf[l]  = sum_{k in path(l)} dir_k * zb_k  -  sum_{k in path(l)} softplus(zb_k)
  leaf_prob    = exp(log_leaf)
  out[b,c]     = sum_t 2*softmax(tw)_t * sum_l leaf_prob[t,l] * softmax(leaf_logits[t,l])_c

Mapping: data-parallel over the batch (B=4096 -> 512 rows per NeuronCore).
Per core, logits live in [tree-node (padded 64/tree), batch] layout so the
per-tree path sums become 128-wide matmuls with +/-1 constant matrices
(block-diagonal over a pair of trees per 128-partition tile). Stage-1/4
matmuls run in bf16, the log-domain path-sum matmul in float32r. softplus
is computed as Ln(Exp(x)+1) so the whole kernel needs a single ACT function
table (pinned up front - the automatic table chooser would otherwise reload
tables between Exp and Ln constantly).
"""

import numpy as np

TREE_DEPTH = 6
T, N, D, C = 64, 63, 512, 100
L = 2**TREE_DEPTH          # 64
NPAD = 64                  # nodes padded per tree
TNP = T * NPAD             # 4096
NTILES = TNP // 128        # 32 (two trees per 128-partition tile)
B = 4096
NCORES = 8
BS = B // NCORES           # 512

# column layout of the packed constants tensor [128, 512]
_COL_BIAS = 0      # [128, 32]
_COL_ADIR = 32     # [128, 128]
_COL_APATH = 160   # [128, 128]
_COL_A2 = 288      # [64, 128]
_COL_E2 = 416      # [64, 32]
_COL_TW = 448      # [1, 64]
_CONST_COLS = 512


def _leaf_paths(depth):
    Ll = 2**depth
    idx = np.zeros((Ll, depth), np.int32)
    dr = np.zeros((Ll, depth), np.int32)
    for l in range(Ll):
        node = 0
        for k in range(depth):
            bit = (l >> (depth - 1 - k)) & 1
            idx[l, k] = node
            dr[l, k] = bit
            node = 2 * node + 1 + bit
    return idx, dr


def _pack_consts(split_bias, tree_weights):
    """Build the [128, 512] packed constants array (f32 bits)."""
    idx, dr = _leaf_paths(TREE_DEPTH)
    mdir = np.zeros((NPAD, L), np.float32)   # [node, leaf] +1 where dir=1
    mpath = np.zeros((NPAD, L), np.float32)  # [node, leaf] -1 on path
    for l in range(L):
        for k in range(TREE_DEPTH):
            n = idx[l, k]
            mpath[n, l] -= 1.0
            if dr[l, k]:
                mdir[n, l] += 1.0
    consts = np.zeros((128, _CONST_COLS), np.float32)
    # bias columns: bias_pad flattened [(tile, partition)] -> [128, 32]
    bpad = np.zeros((T, NPAD), np.float32)
    bpad[:, :N] = split_bias
    consts[:, _COL_BIAS:_COL_BIAS + NTILES] = bpad.reshape(NTILES, 128).T
    # block-diagonal path matrices (two trees per 128-tile)
    consts[:NPAD, _COL_ADIR:_COL_ADIR + L] = mdir
    consts[NPAD:, _COL_ADIR + L:_COL_ADIR + 128] = mdir
    consts[:NPAD, _COL_APATH:_COL_APATH + L] = mpath
    consts[NPAD:, _COL_APATH + L:_COL_APATH + 128] = mpath
    # a2[t, p] = 1 if (t % 2) == p // 64 — broadcast selector for w2 columns
    a2 = np.zeros((T, 128), np.float32)
    for t in range(T):
        a2[t, (t % 2) * 64:(t % 2) * 64 + 64] = 1.0
    consts[:T, _COL_A2:_COL_A2 + 128] = a2
    # e2[t, i] = 1 if t // 2 == i
    e2 = np.zeros((T, NTILES), np.float32)
    for t in range(T):
        e2[t, t // 2] = 1.0
    consts[:T, _COL_E2:_COL_E2 + NTILES] = e2
    consts[0, _COL_TW:_COL_TW + T] = tree_weights
    return consts


_NC_CACHE = {}


def _build_bass():
    import concourse.bacc as bacc
    import concourse.mybir as mybir
    import concourse.tile as tile
    from concourse.hw_specs import get_activation_tables
    from concourse.masks import make_identity

    dt = mybir.dt
    f32 = dt.float32
    f32r = dt.float32r
    bf16 = dt.bfloat16
    AF = mybir.ActivationFunctionType
    ALU = mybir.AluOpType
    AX = mybir.AxisListType

    nc = bacc.Bacc("TRN2", target_bir_lowering=False, debug=False,
                   num_devices=NCORES)

    # Pin the ACT function table to one containing BOTH Exp and Ln, else the
    # table-load pass ping-pongs between single-function tables (~1.3us per
    # reload, one per activation).
    table_id = next(i for i, (_, funcs) in
                    enumerate(get_activation_tables("gen3").items())
                    if AF.Exp in funcs and AF.Ln in funcs)
    nc.scalar.add_instruction(mybir.InstLoadActFuncSet(
        name=f"I-{nc.next_id()}", ins=[], outs=[], act_func_set_id=table_id))

    xt = nc.dram_tensor("xt", [D, BS], bf16, kind="ExternalInput").ap()
    wt = nc.dram_tensor("wt", [D, TNP], bf16, kind="ExternalInput").ap()
    consts = nc.dram_tensor("consts", [128, _CONST_COLS], f32r,
                            kind="ExternalInput").ap()
    llf = nc.dram_tensor("llf", [TNP, C], f32, kind="ExternalInput").ap()
    out = nc.dram_tensor("out", [C, BS], f32, kind="ExternalOutput").ap()

    with tile.TileContext(nc) as tc:
        with (
            tc.tile_pool(name="big", bufs=1) as bigp,
            tc.tile_pool(name="const", bufs=1) as constp,
            tc.tile_pool(name="work", bufs=3) as work,
            tc.tile_pool(name="tmp", bufs=2) as tmpp,
            tc.tile_pool(name="ps", bufs=2, space="PSUM") as psp,
            tc.tile_pool(name="ps1", bufs=1, space="PSUM") as ps1,
        ):
            # ---- input loads, ordered for earliest PE start -----------
            wt_t = [bigp.tile([128, TNP], bf16, tag=f"wt{j}", name=f"wt{j}")
                    for j in range(4)]
            xt_t = bigp.tile([128, 4 * BS], bf16, tag="xt")
            consts_t = constp.tile([128, _CONST_COLS], f32r, tag="consts")
            ll_t = bigp.tile([128, NTILES * C], f32, tag="ll")
            nc.sync.dma_start(out=wt_t[0][:], in_=wt[0:128, :])
            nc.sync.dma_start(out=consts_t[:], in_=consts[:])
            nc.sync.dma_start(
                out=xt_t[:].rearrange("p (j b) -> p j b", b=BS),
                in_=xt.rearrange("(j p) b -> p j b", p=128),
            )
            for j in range(1, 4):
                nc.gpsimd.dma_start(out=wt_t[j][:],
                                    in_=wt[j * 128:(j + 1) * 128, :])
            nc.gpsimd.dma_start(
                out=ll_t[:].rearrange("p (i c) -> p i c", c=C),
                in_=llf.rearrange("(i p) c -> p i c", p=128),
            )

            adir_ap = consts_t[:, _COL_ADIR:_COL_ADIR + 128]
            apath_ap = consts_t[:, _COL_APATH:_COL_APATH + 128]
            a2_ap = consts_t[0:T, _COL_A2:_COL_A2 + 128].bitcast(f32)
            e2_ap = consts_t[0:T, _COL_E2:_COL_E2 + NTILES].bitcast(f32)
            tw_ap = consts_t[0:1, _COL_TW:_COL_TW + T].bitcast(f32)

            def bias_ap(i):
                return consts_t[:, _COL_BIAS + i:_COL_BIAS + i + 1].bitcast(f32)

            ident = constp.tile([64, 64], f32, tag="ident")
            make_identity(nc, ident[:])

            # ---- tree-weight softmax -> per-partition scale columns ----
            mneg = constp.tile([1, 1], f32, tag="mneg")
            nc.vector.tensor_reduce(out=mneg[:], in_=tw_ap, op=ALU.max,
                                    axis=AX.X, negate=True)
            ew = constp.tile([1, T], f32, tag="ew")
            nc.scalar.activation(ew[:], tw_ap, AF.Exp, bias=mneg[:, 0:1],
                                 scale=1.0)
            sw = constp.tile([1, 1], f32, tag="sw")
            nc.vector.tensor_reduce(out=sw[:], in_=ew[:], op=ALU.add, axis=AX.X)
            rw = constp.tile([1, 1], f32, tag="rw")
            nc.vector.reciprocal(rw[:], sw[:])
            wrow = constp.tile([1, T], f32, tag="wrow")
            nc.vector.tensor_scalar(out=wrow[:], in0=ew[:], scalar1=rw[:, 0:1],
                                    scalar2=2.0, op0=ALU.mult, op1=ALU.mult)
            wcol_ps = ps1.tile([T, 1], f32, tag="wcolps")
            nc.tensor.transpose(wcol_ps[:], wrow[:], ident[0:1, 0:1])
            wcol = constp.tile([T, 1], f32, tag="wcol")
            nc.vector.tensor_copy(out=wcol[:], in_=wcol_ps[:])
            bmat = constp.tile([T, NTILES], f32, tag="bmat")
            nc.vector.tensor_scalar_mul(bmat[:], e2_ap, wcol[:, 0:1])
            w2_ps = ps1.tile([128, NTILES], f32, tag="w2ps")
            nc.tensor.matmul(w2_ps[:], lhsT=a2_ap, rhs=bmat[:],
                             start=True, stop=True)
            w2c = constp.tile([128, NTILES], f32, tag="w2c")
            nc.vector.tensor_copy(out=w2c[:], in_=w2_ps[:])

            # ---- leaf distributions: one big exp + rowsum ------------
            ev_all = bigp.tile([128, NTILES * C], f32, tag="evall")
            nc.scalar.activation(ev_all[:], ll_t[:], AF.Exp)
            sv_all = constp.tile([128, NTILES], f32, tag="svall")
            nc.vector.tensor_reduce(
                out=sv_all[:],
                in_=ev_all[:].rearrange("p (i c) -> p i c", c=C),
                op=ALU.add, axis=AX.X)
            rv_all = constp.tile([128, NTILES], f32, tag="rvall")
            nc.vector.reciprocal(rv_all[:], sv_all[:])

            # ---- main pipeline (two 128-tiles per step) --------------
            out_ps = ps1.tile([C, BS], f32, tag="outps")
            ta2 = tb2 = None
            for i in range(NTILES):
                pz = psp.tile([128, BS], f32, tag="pz")
                for j in range(4):
                    nc.tensor.matmul(
                        pz[:],
                        lhsT=wt_t[j][:, i * 128:(i + 1) * 128],
                        rhs=xt_t[:, j * BS:(j + 1) * BS],
                        start=(j == 0), stop=(j == 3),
                    )
                if i % 2 == 0:
                    ta2 = work.tile([128, 2 * BS], f32r, tag="ta2")
                    tb2 = work.tile([128, 2 * BS], f32r, tag="tb2")
                half = (i % 2) * BS
                ta = ta2[:, half:half + BS]
                nc.vector.tensor_scalar_add(out=ta, in0=pz[:],
                                            scalar1=bias_ap(i))
                if i % 2 == 1:
                    # one Exp + one Ln covering both halves (SBUF source)
                    te = tmpp.tile([128, 2 * BS], f32, tag="te")
                    nc.scalar.activation(te[:], ta2[:].bitcast(f32), AF.Exp)
                    nc.scalar.activation(tb2[:], te[:], AF.Ln, bias=1.0,
                                         scale=1.0)
                    for h in range(2):
                        ii = i - 1 + h
                        pp = psp.tile([128, BS], f32, tag="pp")
                        nc.tensor.matmul(pp[:], lhsT=adir_ap,
                                         rhs=ta2[:, h * BS:(h + 1) * BS],
                                         start=True, stop=False)
                        nc.tensor.matmul(pp[:], lhsT=apath_ap,
                                         rhs=tb2[:, h * BS:(h + 1) * BS],
                                         start=False, stop=True)
                        lp = work.tile([128, BS], bf16, tag="lp")
                        nc.scalar.activation(lp[:], pp[:], AF.Exp)
                        vt = work.tile([128, C], bf16, tag="vt")
                        nc.vector.tensor_scalar(
                            out=vt[:],
                            in0=ev_all[:, ii * C:(ii + 1) * C],
                            scalar1=rv_all[:, ii:ii + 1],
                            scalar2=w2c[:, ii:ii + 1],
                            op0=ALU.mult, op1=ALU.mult)
                        nc.tensor.matmul(out_ps[:], lhsT=vt[:], rhs=lp[:],
                                         start=(ii == 0),
                                         stop=(ii == NTILES - 1))

            out_sb = work.tile([C, BS], f32, tag="osb")
            nc.vector.tensor_copy(out=out_sb[:], in_=out_ps[:])
            nc.sync.dma_start(out=out[:], in_=out_sb[:])

    nc.finalize()
    return nc


def _get_nc():
    if "nc" not in _NC_CACHE:
        _NC_CACHE["nc"] = _build_bass()
    return _NC_CACHE["nc"]


def _prep_inputs(x, split_weights, split_bias, leaf_logits, tree_weights):
    import ml_dtypes

    x = np.asarray(x, np.float32)
    split_weights = np.asarray(split_weights, np.float32)
    split_bias = np.asarray(split_bias, np.float32)
    leaf_logits = np.asarray(leaf_logits, np.float32)
    tree_weights = np.asarray(tree_weights, np.float32)

    wpad = np.zeros((T, NPAD, D), np.float32)
    wpad[:, :N, :] = split_weights
    wtT = np.ascontiguousarray(
        wpad.reshape(TNP, D).T.astype(ml_dtypes.bfloat16))      # [D, TNP]
    consts = _pack_consts(split_bias, tree_weights)
    llf = np.ascontiguousarray(leaf_logits.reshape(TNP, C))

    shared = dict(wt=wtT, consts=consts, llf=llf)
    in_maps = []
    for i in range(NCORES):
        xti = np.ascontiguousarray(
            x[i * BS:(i + 1) * BS, :].T.astype(ml_dtypes.bfloat16))  # [D, BS]
        in_maps.append(dict(xt=xti, **shared))
    return in_maps


def kernel(x, split_weights, split_bias, leaf_logits, tree_weights):
    from concourse.bass_utils import run_bass_kernel_spmd

    in_maps = _prep_inputs(x, split_weights, split_bias, leaf_logits,
                           tree_weights)
    nc = _get_nc()
    res = run_bass_kernel_spmd(nc, in_maps, core_ids=list(range(NCORES)))
    out = np.concatenate([res.results[i]["out"] for i in range(NCORES)],
                         axis=1).T                              # [B, C]
    return np.ascontiguousarray(out.astype(np.float32))


# revision 13
# speedup vs baseline: 1.0685x; 1.0096x over previous
"""Trainium2 Bass kernel for the soft-decision-tree ensemble classifier.

Math (per batch row b, tree t):
  zb[t,n]      = x[b] . W[t,n] + bias[t,n]
  log s        = zb - softplus(zb);  log(1-s) = -softplus(zb)
  log_leaf[l]  = sum_{k in path(l)} dir_k * zb_k  -  sum_{k in path(l)} softplus(zb_k)
  leaf_prob    = exp(log_leaf)
  out[b,c]     = sum_t 2*softmax(tw)_t * sum_l leaf_prob[t,l] * softmax(leaf_logits[t,l])_c

Mapping: data-parallel over the batch (B=4096 -> 512 rows per NeuronCore).
Per core, logits live in [tree-node (padded 64/tree), batch] layout so the
per-tree path sums become 128-wide matmuls with +/-1 constant matrices
(block-diagonal over a pair of trees per 128-partition tile). Stage-1/4
matmuls run in bf16, the log-domain path-sum matmul in float32r. softplus
is computed as Ln(Exp(x)+1) so the whole kernel needs a single ACT function
table (pinned up front - the automatic table chooser would otherwise reload
tables between Exp and Ln constantly).
"""

import numpy as np

TREE_DEPTH = 6
T, N, D, C = 64, 63, 512, 100
L = 2**TREE_DEPTH          # 64
NPAD = 64                  # nodes padded per tree
TNP = T * NPAD             # 4096
NTILES = TNP // 128        # 32 (two trees per 128-partition tile)
B = 4096
NCORES = 8
BS = B // NCORES           # 512

# column layout of the packed constants tensor [128, 512]
_COL_BIAS = 0      # [128, 32]
_COL_ADIR = 32     # [128, 128]
_COL_APATH = 160   # [128, 128]
_COL_A2 = 288      # [64, 128]
_COL_E2 = 416      # [64, 32]
_COL_TW = 448      # [1, 64]
_CONST_COLS = 512


def _leaf_paths(depth):
    Ll = 2**depth
    idx = np.zeros((Ll, depth), np.int32)
    dr = np.zeros((Ll, depth), np.int32)
    for l in range(Ll):
        node = 0
        for k in range(depth):
            bit = (l >> (depth - 1 - k)) & 1
            idx[l, k] = node
            dr[l, k] = bit
            node = 2 * node + 1 + bit
    return idx, dr


def _pack_consts(split_bias, tree_weights):
    """Build the [128, 512] packed constants array (f32 bits)."""
    idx, dr = _leaf_paths(TREE_DEPTH)
    mdir = np.zeros((NPAD, L), np.float32)   # [node, leaf] +1 where dir=1
    mpath = np.zeros((NPAD, L), np.float32)  # [node, leaf] -1 on path
    for l in range(L):
        for k in range(TREE_DEPTH):
            n = idx[l, k]
            mpath[n, l] -= 1.0
            if dr[l, k]:
                mdir[n, l] += 1.0
    consts = np.zeros((128, _CONST_COLS), np.float32)
    # bias columns: bias_pad flattened [(tile, partition)] -> [128, 32]
    bpad = np.zeros((T, NPAD), np.float32)
    bpad[:, :N] = split_bias
    consts[:, _COL_BIAS:_COL_BIAS + NTILES] = bpad.reshape(NTILES, 128).T
    # block-diagonal path matrices (two trees per 128-tile)
    consts[:NPAD, _COL_ADIR:_COL_ADIR + L] = mdir
    consts[NPAD:, _COL_ADIR + L:_COL_ADIR + 128] = mdir
    consts[:NPAD, _COL_APATH:_COL_APATH + L] = mpath
    consts[NPAD:, _COL_APATH + L:_COL_APATH + 128] = mpath
    # a2[t, p] = 1 if (t % 2) == p // 64 — broadcast selector for w2 columns
    a2 = np.zeros((T, 128), np.float32)
    for t in range(T):
        a2[t, (t % 2) * 64:(t % 2) * 64 + 64] = 1.0
    consts[:T, _COL_A2:_COL_A2 + 128] = a2
    # e2[t, i] = 1 if t // 2 == i
    e2 = np.zeros((T, NTILES), np.float32)
    for t in range(T):
        e2[t, t // 2] = 1.0
    consts[:T, _COL_E2:_COL_E2 + NTILES] = e2
    consts[0, _COL_TW:_COL_TW + T] = tree_weights
    return consts


_NC_CACHE = {}


def _build_bass():
    import concourse.bacc as bacc
    import concourse.mybir as mybir
    import concourse.tile as tile
    from concourse.hw_specs import get_activation_tables
    from concourse.masks import make_identity

    dt = mybir.dt
    f32 = dt.float32
    f32r = dt.float32r
    bf16 = dt.bfloat16
    AF = mybir.ActivationFunctionType
    ALU = mybir.AluOpType
    AX = mybir.AxisListType

    nc = bacc.Bacc("TRN2", target_bir_lowering=False, debug=False,
                   num_devices=NCORES)

    # Pin the ACT function table to one containing BOTH Exp and Ln, else the
    # table-load pass ping-pongs between single-function tables (~1.3us per
    # reload, one per activation).
    table_id = next(i for i, (_, funcs) in
                    enumerate(get_activation_tables("gen3").items())
                    if AF.Exp in funcs and AF.Ln in funcs)
    nc.scalar.add_instruction(mybir.InstLoadActFuncSet(
        name=f"I-{nc.next_id()}", ins=[], outs=[], act_func_set_id=table_id))

    xt = nc.dram_tensor("xt", [D, BS], bf16, kind="ExternalInput").ap()
    wt = nc.dram_tensor("wt", [D, TNP], bf16, kind="ExternalInput").ap()
    consts = nc.dram_tensor("consts", [128, _CONST_COLS], f32r,
                            kind="ExternalInput").ap()
    llf = nc.dram_tensor("llf", [TNP, C], f32, kind="ExternalInput").ap()
    out = nc.dram_tensor("out", [C, BS], f32, kind="ExternalOutput").ap()

    with tile.TileContext(nc) as tc:
        with (
            tc.tile_pool(name="big", bufs=1) as bigp,
            tc.tile_pool(name="const", bufs=1) as constp,
            tc.tile_pool(name="work", bufs=3) as work,
            tc.tile_pool(name="tmp", bufs=2) as tmpp,
            tc.tile_pool(name="ps", bufs=2, space="PSUM") as psp,
            tc.tile_pool(name="ps1", bufs=1, space="PSUM") as ps1,
        ):
            # ---- input loads, ordered for earliest PE start -----------
            wt_t = [bigp.tile([128, TNP], bf16, tag=f"wt{j}", name=f"wt{j}")
                    for j in range(4)]
            xt_t = bigp.tile([128, 4 * BS], bf16, tag="xt")
            consts_t = constp.tile([128, _CONST_COLS], f32r, tag="consts")
            ll_t = bigp.tile([128, NTILES * C], f32, tag="ll")
            nc.sync.dma_start(out=wt_t[0][:], in_=wt[0:128, :])
            nc.sync.dma_start(out=consts_t[:], in_=consts[:])
            nc.sync.dma_start(
                out=xt_t[:].rearrange("p (j b) -> p j b", b=BS),
                in_=xt.rearrange("(j p) b -> p j b", p=128),
            )
            nc.scalar.dma_start(out=wt_t[1][:], in_=wt[128:256, :])
            nc.scalar.dma_start(out=wt_t[2][:], in_=wt[256:384, :])
            nc.sync.dma_start(out=wt_t[3][:], in_=wt[384:512, :])
            nc.sync.dma_start(
                out=ll_t[:].rearrange("p (i c) -> p i c", c=C),
                in_=llf.rearrange("(i p) c -> p i c", p=128),
            )

            ident = constp.tile([64, 64], f32, tag="ident")
            make_identity(nc, ident[:])

            adir_ap = consts_t[:, _COL_ADIR:_COL_ADIR + 128]
            apath_ap = consts_t[:, _COL_APATH:_COL_APATH + 128]
            a2_ap = consts_t[0:T, _COL_A2:_COL_A2 + 128].bitcast(f32)
            e2_ap = consts_t[0:T, _COL_E2:_COL_E2 + NTILES].bitcast(f32)
            tw_ap = consts_t[0:1, _COL_TW:_COL_TW + T].bitcast(f32)

            def bias_ap(i):
                return consts_t[:, _COL_BIAS + i:_COL_BIAS + i + 1].bitcast(f32)

            # ---- tree-weight softmax -> per-partition scale columns ----
            mneg = constp.tile([1, 1], f32, tag="mneg")
            nc.vector.tensor_reduce(out=mneg[:], in_=tw_ap, op=ALU.max,
                                    axis=AX.X, negate=True)
            ew = constp.tile([1, T], f32, tag="ew")
            nc.scalar.activation(ew[:], tw_ap, AF.Exp, bias=mneg[:, 0:1],
                                 scale=1.0)
            sw = constp.tile([1, 1], f32, tag="sw")
            nc.vector.tensor_reduce(out=sw[:], in_=ew[:], op=ALU.add, axis=AX.X)
            rw = constp.tile([1, 1], f32, tag="rw")
            nc.vector.reciprocal(rw[:], sw[:])
            wrow = constp.tile([1, T], f32, tag="wrow")
            nc.vector.tensor_scalar(out=wrow[:], in0=ew[:], scalar1=rw[:, 0:1],
                                    scalar2=2.0, op0=ALU.mult, op1=ALU.mult)
            wcol_ps = ps1.tile([T, 1], f32, tag="wcolps")
            nc.tensor.transpose(wcol_ps[:], wrow[:], ident[0:1, 0:1])
            wcol = constp.tile([T, 1], f32, tag="wcol")
            nc.vector.tensor_copy(out=wcol[:], in_=wcol_ps[:])
            bmat = constp.tile([T, NTILES], f32, tag="bmat")
            nc.vector.tensor_scalar_mul(bmat[:], e2_ap, wcol[:, 0:1])
            w2_ps = ps1.tile([128, NTILES], f32, tag="w2ps")
            nc.tensor.matmul(w2_ps[:], lhsT=a2_ap, rhs=bmat[:],
                             start=True, stop=True)
            w2c = constp.tile([128, NTILES], f32, tag="w2c")
            nc.vector.tensor_copy(out=w2c[:], in_=w2_ps[:])

            # ---- leaf distributions: one big exp + rowsum ------------
            ev_all = bigp.tile([128, NTILES * C], f32, tag="evall")
            nc.scalar.activation(ev_all[:], ll_t[:], AF.Exp)
            sv_all = constp.tile([128, NTILES], f32, tag="svall")
            nc.vector.tensor_reduce(
                out=sv_all[:],
                in_=ev_all[:].rearrange("p (i c) -> p i c", c=C),
                op=ALU.add, axis=AX.X)
            rv_all = constp.tile([128, NTILES], f32, tag="rvall")
            nc.vector.reciprocal(rv_all[:], sv_all[:])

            # ---- main pipeline (two 128-tiles per step) --------------
            out_ps = ps1.tile([C, BS], f32, tag="outps")
            ta2 = tb2 = None
            for i in range(NTILES):
                pz = psp.tile([128, BS], f32, tag="pz")
                for j in range(4):
                    nc.tensor.matmul(
                        pz[:],
                        lhsT=wt_t[j][:, i * 128:(i + 1) * 128],
                        rhs=xt_t[:, j * BS:(j + 1) * BS],
                        start=(j == 0), stop=(j == 3),
                    )
                if i % 2 == 0:
                    ta2 = work.tile([128, 2 * BS], f32r, tag="ta2")
                    tb2 = work.tile([128, 2 * BS], f32r, tag="tb2")
                half = (i % 2) * BS
                ta = ta2[:, half:half + BS]
                nc.vector.tensor_scalar_add(out=ta, in0=pz[:],
                                            scalar1=bias_ap(i))
                if i % 2 == 1:
                    # one Exp + one Ln covering both halves (SBUF source)
                    te = tmpp.tile([128, 2 * BS], f32, tag="te")
                    nc.scalar.activation(te[:], ta2[:].bitcast(f32), AF.Exp)
                    nc.scalar.activation(tb2[:], te[:], AF.Ln, bias=1.0,
                                         scale=1.0)
                    for h in range(2):
                        ii = i - 1 + h
                        pp = psp.tile([128, BS], f32, tag="pp")
                        nc.tensor.matmul(pp[:], lhsT=adir_ap,
                                         rhs=ta2[:, h * BS:(h + 1) * BS],
                                         start=True, stop=False)
                        nc.tensor.matmul(pp[:], lhsT=apath_ap,
                                         rhs=tb2[:, h * BS:(h + 1) * BS],
                                         start=False, stop=True)
                        lp = work.tile([128, BS], bf16, tag="lp")
                        nc.scalar.activation(lp[:], pp[:], AF.Exp)
                        vt = work.tile([128, C], bf16, tag="vt")
                        nc.vector.tensor_scalar(
                            out=vt[:],
                            in0=ev_all[:, ii * C:(ii + 1) * C],
                            scalar1=rv_all[:, ii:ii + 1],
                            scalar2=w2c[:, ii:ii + 1],
                            op0=ALU.mult, op1=ALU.mult)
                        nc.tensor.matmul(out_ps[:], lhsT=vt[:], rhs=lp[:],
                                         start=(ii == 0),
                                         stop=(ii == NTILES - 1))

            out_sb = work.tile([C, BS], f32, tag="osb")
            nc.vector.tensor_copy(out=out_sb[:], in_=out_ps[:])
            nc.sync.dma_start(out=out[:], in_=out_sb[:])

    nc.finalize()
    return nc


def _get_nc():
    if "nc" not in _NC_CACHE:
        _NC_CACHE["nc"] = _build_bass()
    return _NC_CACHE["nc"]


def _prep_inputs(x, split_weights, split_bias, leaf_logits, tree_weights):
    import ml_dtypes

    x = np.asarray(x, np.float32)
    split_weights = np.asarray(split_weights, np.float32)
    split_bias = np.asarray(split_bias, np.float32)
    leaf_logits = np.asarray(leaf_logits, np.float32)
    tree_weights = np.asarray(tree_weights, np.float32)

    wpad = np.zeros((T, NPAD, D), np.float32)
    wpad[:, :N, :] = split_weights
    wtT = np.ascontiguousarray(
        wpad.reshape(TNP, D).T.astype(ml_dtypes.bfloat16))      # [D, TNP]
    consts = _pack_consts(split_bias, tree_weights)
    llf = np.ascontiguousarray(leaf_logits.reshape(TNP, C))

    shared = dict(wt=wtT, consts=consts, llf=llf)
    in_maps = []
    for i in range(NCORES):
        xti = np.ascontiguousarray(
            x[i * BS:(i + 1) * BS, :].T.astype(ml_dtypes.bfloat16))  # [D, BS]
        in_maps.append(dict(xt=xti, **shared))
    return in_maps


def kernel(x, split_weights, split_bias, leaf_logits, tree_weights):
    from concourse.bass_utils import run_bass_kernel_spmd

    in_maps = _prep_inputs(x, split_weights, split_bias, leaf_logits,
                           tree_weights)
    nc = _get_nc()
    res = run_bass_kernel_spmd(nc, in_maps, core_ids=list(range(NCORES)))
    out = np.concatenate([res.results[i]["out"] for i in range(NCORES)],
                         axis=1).T                              # [B, C]
    return np.ascontiguousarray(out.astype(np.float32))
